# revision 11
# baseline (speedup 1.0000x reference)
"""Causal multi-head self-attention with RoPE on 8 Trainium2 NeuronCores.

Problem: x[2, 2048, 1024] fp32, 16 heads, d_head=64, causal, RoPE(theta=1e4).
Sharding: core = b*4 + g  (b in {0,1} batch, g in {0..3} head-group of 4 heads).
Each core computes out_partial[2048, 1024] = attn(heads of g) @ wo[:, cols_g].T
in bf16; host sums the 4 partials per batch in fp32.

v2 layout (vs v1): multi-queue input DMA; ko-outer Q projection pipelined
against the sequential x-chunk arrivals; qc-outer main loop with the output
projection fused per q-chunk; causal narrowing of scores/exp/attn on diagonal
k-tiles; bf16 output; denominator broadcast bounce on the gpsimd DMA queue.
"""

import os
import sys

sys.path.insert(0, "/opt/trn_rl_repo")

import ml_dtypes
import numpy as np

import concourse.bacc as bacc
import concourse.mybir as mybir
from concourse.tile import TileContext

B = 2
S = 2048
DM = 1024
H = 16
DH = 64
HLOC = 4  # heads per core
SC = 512  # q chunk size
NKT = S // 128  # 16 k tiles
NQC = S // SC  # 4 q chunks
P = 128
KO = DM // P  # 8 contraction subtiles for projections
SCALE = 1.0 / 8.0  # 1/sqrt(DH)
THETA = 10000.0

F32 = mybir.dt.float32
BF16 = mybir.dt.bfloat16

_CACHE = {}
DEBUG = False


def _build_nc():
    nc = bacc.Bacc("TRN2", enable_partition_id=False)
    Exp = mybir.ActivationFunctionType.Exp

    xT = nc.dram_tensor("xT", [DM, S], BF16, kind="ExternalInput")
    wq_t = nc.dram_tensor("wq_t", [DM, 256], BF16, kind="ExternalInput")
    wk_t = nc.dram_tensor("wk_t", [DM, 256], BF16, kind="ExternalInput")
    wv_t = nc.dram_tensor("wv_t", [DM, 256], BF16, kind="ExternalInput")
    wo_t = nc.dram_tensor("wo_t", [256, DM], BF16, kind="ExternalInput")
    cosT = nc.dram_tensor("cosT", [P, S], BF16, kind="ExternalInput")
    sinT = nc.dram_tensor("sinT", [P, S], BF16, kind="ExternalInput")
    perm = nc.dram_tensor("perm", [P, P], BF16, kind="ExternalInput")
    tri = nc.dram_tensor("tri", [P, 2, P], BF16, kind="ExternalInput")
    outp = nc.dram_tensor("out_partial", [S, DM], BF16, kind="ExternalOutput")

    with TileContext(nc) as tc:
        with tc.tile_pool(name="persist", bufs=1) as persist, \
             tc.tile_pool(name="mw", bufs=1) as mw, \
             tc.tile_pool(name="mp", bufs=6) as mp, \
             tc.tile_pool(name="mn", bufs=2) as mn, \
             tc.tile_pool(name="mo", bufs=2) as mo, \
             tc.tile_pool(name="mdr", bufs=4, space="DRAM") as mdr:
            # [pair-head-dim (2*64), head-pair, seq]
            q_rot = persist.tile([P, 2, S], BF16, tag="q_rot")
            k_rot = persist.tile([P, 2, S], BF16, tag="k_rot")
            # V in [k partitions, k_tile, head, 72]: cols 0:64 = V, 64 = ones
            v_sb = persist.tile([P, NKT, HLOC, 72], BF16, tag="v_sb")
            # attention output, transposed: [head-dim rows, head-pair, seq]
            attnT = persist.tile([P, 2, S], BF16, tag="attnT")

            wo_sb = mw.tile([P, 2, DM], BF16, tag="wo_sb")
            tri_sb = mw.tile([P, 2, P], BF16, tag="tri_sb")
            # x + projection weights stay resident: projection chains for
            # chunk qc+1 run inside chunk qc's attention window
            xT_sb = mw.tile([P, KO, S], BF16, tag="xT_sb")
            wq_sb = mw.tile([P, KO, 256], BF16, tag="wq_sb")
            wk_sb = mw.tile([P, KO, 256], BF16, tag="wk_sb")
            wv_sb = mw.tile([P, KO, 256], BF16, tag="wv_sb")
            cos_sb = mw.tile([P, S], BF16, tag="cos_sb")
            sin_sb = mw.tile([P, S], BF16, tag="sin_sb")
            perm_sb = mw.tile([P, P], BF16, tag="perm_sb")

            # weights + tables on the scalar engine's DMA queue (per-ko
            # chunks so wq[ko0] lands before the first projection round)
            for t, dt_ in ((wq_sb, wq_t), (wk_sb, wk_t), (wv_sb, wv_t)):
                d_ap = dt_[:].rearrange("(ko p) m -> p ko m", p=P)
                for ko in range(KO):
                    nc.scalar.dma_start(t[:, ko, :], d_ap[:, ko, :])
            nc.scalar.dma_start(cos_sb[:], cosT[:])
            nc.scalar.dma_start(sin_sb[:], sinT[:])
            nc.scalar.dma_start(perm_sb[:], perm[:])
            # x chunks sequentially on the sync queue: chunk ko arrives
            # ~1.6us apart, pacing the ko-outer projection rounds
            xT_ap = xT[:].rearrange("(ko p) s -> p ko s", p=P)
            for ko in range(KO):
                nc.sync.dma_start(xT_sb[:, ko, :], xT_ap[:, ko, :])
            nc.sync.dma_start(
                wo_sb[:], wo_t[:].rearrange("(ko p) m -> p ko m", p=P)
            )
            nc.sync.dma_start(tri_sb[:], tri[:])

            # ones column for the denominator trick
            nc.vector.memset(v_sb[:, :, :, 64:65], 1.0)

            def rope(pool, tag, a_ps, dest, cs_):
                # dest = a*cos + P@(a*sin); a staged to bf16 once on ACT.
                # b_ps reuses the slot a_ps occupied (freed by the copy).
                a_sb = mn.tile([P, SC], BF16, tag="a_sb", bufs=3)
                nc.scalar.copy(out=a_sb[:], in_=a_ps[:])
                t2 = mn.tile([P, SC], BF16, tag="t2", bufs=2)
                nc.vector.tensor_mul(
                    out=t2[:], in0=a_sb[:], in1=sin_sb[:, cs_]
                )
                b_ps = pool.tile([P, SC], F32, tag=tag, name="b_ps")
                nc.tensor.matmul(
                    b_ps[:], lhsT=perm_sb[:], rhs=t2[:],
                    start=True, stop=True,
                )
                nc.vector.tensor_mul(
                    out=dest, in0=a_sb[:], in1=cos_sb[:, cs_]
                )
                nc.vector.tensor_add(out=dest, in0=dest, in1=b_ps[:])

            def v_copy(v_ps, st):
                nc.vector.tensor_copy(
                    out=v_sb[:, st, :, 0:64],
                    in_=v_ps[:].rearrange("p (h d) -> p h d", d=DH),
                )

            # ---------------- Phase B: chunk-0 projections ------------------
            # Only what attention chunk 0 needs: Q/K of qc0 (both hp) and
            # V st0-3, ko-outer across 8 PSUM tiles so the PE follows the
            # x-chunk DMA stream; later chunks project inside the main loop.
            cs0 = slice(0, SC)
            with tc.tile_pool(name="bp", bufs=8, space="PSUM") as bp:
                qa = [bp.tile([P, SC], F32, tag="proj", name=f"qa{hp}")
                      for hp in range(2)]
                ka = [bp.tile([P, SC], F32, tag="proj", name=f"ka{hp}")
                      for hp in range(2)]
                va = [bp.tile([P, 256], F32, tag="proj", name=f"va{st}")
                      for st in range(4)]
                for ko in range(KO):
                    for hp in range(2):
                        nc.tensor.matmul(
                            qa[hp][:],
                            lhsT=wq_sb[:, ko, hp * P:(hp + 1) * P],
                            rhs=xT_sb[:, ko, cs0],
                            start=(ko == 0), stop=(ko == KO - 1),
                        )
                    for hp in range(2):
                        nc.tensor.matmul(
                            ka[hp][:],
                            lhsT=wk_sb[:, ko, hp * P:(hp + 1) * P],
                            rhs=xT_sb[:, ko, cs0],
                            start=(ko == 0), stop=(ko == KO - 1),
                        )
                    for st in range(4):
                        nc.tensor.matmul(
                            va[st][:],
                            lhsT=xT_sb[:, ko, st * P:(st + 1) * P],
                            rhs=wv_sb[:, ko, :],
                            start=(ko == 0), stop=(ko == KO - 1),
                        )
                for hp in range(2):
                    rope(bp, "proj", qa[hp], q_rot[:, hp, cs0], cs0)
                for hp in range(2):
                    rope(bp, "proj", ka[hp], k_rot[:, hp, cs0], cs0)
                for st in range(4):
                    v_copy(va[st], st)

            # ---------------- Main loop: attention + proj + out proj --------
            # hp streams interleave at (kp, j) granularity; next chunk's
            # projection chains slot into the ACT-bound stretches, borrowing
            # the scores ring; phase E o_ps borrows the at rings.
            with tc.tile_pool(name="ds", bufs=2, space="PSUM") as dsp, \
                 tc.tile_pool(name="da", bufs=2, space="PSUM") as dap:
                out_ap = outp[:].rearrange("(st p) m -> p st m", p=P)

                def emit_proj(item, qn):
                    csn = slice(qn * SC, (qn + 1) * SC)
                    kind, i = item
                    if kind == "q" or kind == "k":
                        w_sb = wq_sb if kind == "q" else wk_sb
                        dest = q_rot if kind == "q" else k_rot
                        a_ps = dsp.tile([P, SC], F32, tag="scores",
                                        name="a_ps")
                        for ko in range(KO):
                            nc.tensor.matmul(
                                a_ps[:],
                                lhsT=w_sb[:, ko, i * P:(i + 1) * P],
                                rhs=xT_sb[:, ko, csn],
                                start=(ko == 0), stop=(ko == KO - 1),
                            )
                        rope(dsp, "scores", a_ps, dest[:, i, csn], csn)
                    else:
                        v_ps = dsp.tile([P, 256], F32, tag="scores",
                                        name="v_ps")
                        for ko in range(KO):
                            nc.tensor.matmul(
                                v_ps[:],
                                lhsT=xT_sb[:, ko, i * P:(i + 1) * P],
                                rhs=wv_sb[:, ko, :],
                                start=(ko == 0), stop=(ko == KO - 1),
                            )
                        v_copy(v_ps, i)

                for qc in range(NQC):
                    cs = slice(qc * SC, (qc + 1) * SC)
                    nkt_v = 4 * qc + 4
                    if qc < 3:
                        proj_items = [
                            ("q", 0), ("k", 0), ("q", 1), ("k", 1),
                        ] + [("v", st)
                             for st in range(4 * qc + 4, 4 * qc + 8)]
                    else:
                        proj_items = []
                    n_js = nkt_v * 2
                    stride = max(1, n_js // 8)
                    ji = 0
                    at = {}
                    for hp in range(2):
                        for hh in range(2):
                            at[(hp, hh)] = dap.tile(
                                [65, SC], F32, tag=f"at{hh}",
                                name=f"at{hp}{hh}")
                    for kp in range(nkt_v // 2):
                        for hp in range(2):
                            for j in range(2):
                                if proj_items and ji % stride == 0:
                                    emit_proj(proj_items.pop(0), qc + 1)
                                ji += 1
                                kt = 2 * kp + j
                                r = kt - 4 * qc
                                w0 = 128 * r if r > 0 else 0
                                s2 = dsp.tile([P, 2, SC], F32, tag="scores",
                                              name="s2")
                                for hh in range(2):
                                    hs = slice(hh * 64, (hh + 1) * 64)
                                    nc.tensor.matmul(
                                        s2[:, hh, w0:SC],
                                        lhsT=k_rot[hs, hp,
                                                   kt * P:(kt + 1) * P],
                                        rhs=q_rot[hs, hp,
                                                  qc * SC + w0:(qc + 1) * SC],
                                        start=True,
                                        stop=True,
                                    )
                                pt = mp.tile([P, 2, SC], BF16, tag="pt",
                                             name="pt", bufs=6)
                                if w0 > 0:
                                    nc.gpsimd.memset(pt[:, :, 0:w0], 0.0)
                                nc.scalar.activation(
                                    out=pt[:, :, w0:SC],
                                    in_=s2[:, :, w0:SC],
                                    func=Exp, scale=SCALE,
                                )
                                if r >= 0:
                                    nc.vector.tensor_mul(
                                        out=pt[:, :, w0:w0 + 128],
                                        in0=pt[:, :, w0:w0 + 128],
                                        in1=tri_sb[:],
                                    )
                                for hh in range(2):
                                    nc.tensor.matmul(
                                        at[(hp, hh)][:],
                                        lhsT=v_sb[:, kt, 2 * hp + hh, 0:65],
                                        rhs=pt[:, hh, :],
                                        start=(kt == 0),
                                        stop=(kt == nkt_v - 1),
                                    )
                    while proj_items:
                        emit_proj(proj_items.pop(0), qc + 1)
                    # normalize: rows 0:64 are attn, row 64 is denom;
                    # partition-broadcast 1/denom via DRAM bounce
                    for hp in range(2):
                        for hh in range(2):
                            rd = mn.tile([P, SC], F32, tag="rd")
                            nc.vector.tensor_copy(
                                out=rd[64:65, :], in_=at[(hp, hh)][64:65, :]
                            )
                            dr = mdr.tile([1, SC], F32, tag="dr")
                            nc.sync.dma_start(dr[:], rd[64:65, :])
                            den_bc = mn.tile([64, SC], F32, tag="den_bc")
                            nc.sync.dma_start(
                                den_bc[:], dr[:].partition_broadcast(64)
                            )
                            rbc = mn.tile([64, SC], F32, tag="rbc")
                            nc.vector.reciprocal_approx_fast(
                                out=rbc[:], in_=den_bc[:]
                            )
                            if hh == 0:
                                nc.vector.tensor_mul(
                                    out=attnT[0:64, hp, cs],
                                    in0=at[(hp, hh)][0:64, :],
                                    in1=rbc[:],
                                )
                            else:
                                tmp = mn.tile([64, SC], BF16, tag="tmp")
                                nc.vector.tensor_mul(
                                    out=tmp[:], in0=at[(hp, hh)][0:64, :],
                                    in1=rbc[:],
                                )
                                nc.sync.dma_start(
                                    attnT[64:128, hp, cs], tmp[:]
                                )
                    # output projection for this q chunk; o_ps borrows the
                    # at-tag rings so next chunk's scores never wait on it
                    for st in range(4 * qc, 4 * qc + 4):
                        ob = mo.tile([P, DM], BF16, tag="ob")
                        for no in range(2):
                            o_ps = dap.tile([P, SC], F32, tag=f"at{no}",
                                            name="o_ps")
                            for ko in range(2):
                                nc.tensor.matmul(
                                    o_ps[:],
                                    lhsT=attnT[:, ko, st * P:(st + 1) * P],
                                    rhs=wo_sb[:, ko, no * SC:(no + 1) * SC],
                                    start=(ko == 0),
                                    stop=(ko == 1),
                                )
                            osl = ob[:, no * SC:(no + 1) * SC]
                            nc.vector.tensor_copy(out=osl, in_=o_ps[:])
                        nc.sync.dma_start(out_ap[:, st, :], ob[:])
            if DEBUG:
                dq = nc.dram_tensor("dbg_qrot", [P, 2, S], BF16,
                                    kind="ExternalOutput")
                dk = nc.dram_tensor("dbg_krot", [P, 2, S], BF16,
                                    kind="ExternalOutput")
                dv = nc.dram_tensor("dbg_vsb", [P, NKT, HLOC, 72], BF16,
                                    kind="ExternalOutput")
                da_ = nc.dram_tensor("dbg_attnT", [P, 2, S], BF16,
                                     kind="ExternalOutput")
                nc.sync.dma_start(dq[:], q_rot[:])
                nc.sync.dma_start(dk[:], k_rot[:])
                nc.sync.dma_start(dv[:], v_sb[:])
                nc.sync.dma_start(da_[:], attnT[:])
    nc.compile()
    return nc


def _host_tables(token_positions):
    pos = np.asarray(token_positions).astype(np.float64)
    freq = 1.0 / (THETA ** (2.0 * np.arange(DH // 2, dtype=np.float64) / DH))
    ang = pos[:, None] * freq[None, :]  # [S, 32]
    cos_f = np.repeat(np.cos(ang), 2, axis=1)  # [S, 64]
    sin_f = np.repeat(np.sin(ang), 2, axis=1)
    cosT = np.ascontiguousarray(
        np.concatenate([cos_f.T, cos_f.T], axis=0)
    ).astype(ml_dtypes.bfloat16)  # [128, S]
    sinT = np.ascontiguousarray(
        np.concatenate([sin_f.T, sin_f.T], axis=0)
    ).astype(ml_dtypes.bfloat16)

    perm = np.zeros((P, P), dtype=ml_dtypes.bfloat16)
    for i in range(P // 2):
        perm[2 * i + 1, 2 * i] = -1.0
        perm[2 * i, 2 * i + 1] = 1.0

    p_idx = np.arange(P)[:, None]
    f_idx = np.arange(P)[None, :]
    tri1 = (f_idx >= p_idx).astype(ml_dtypes.bfloat16)  # [128, 128]
    tri = np.ascontiguousarray(
        np.broadcast_to(tri1[:, None, :], (P, 2, P))
    )  # [128, 2, 128]
    return cosT, sinT, perm, tri


_LAST_RESULTS = None


def _bf16(a):
    return np.ascontiguousarray(a).astype(ml_dtypes.bfloat16)


def kernel(x, wq, wk, wv, wo, token_positions):
    global _LAST_RESULTS
    from concourse.bass_utils import run_bass_kernel_spmd

    if "nc" not in _CACHE:
        _CACHE["nc"] = _build_nc()
    nc = _CACHE["nc"]

    x = np.asarray(x, dtype=np.float32)
    wq = np.asarray(wq, dtype=np.float32)
    wk = np.asarray(wk, dtype=np.float32)
    wv = np.asarray(wv, dtype=np.float32)
    wo = np.asarray(wo, dtype=np.float32)
    cosT, sinT, perm, tri = _host_tables(token_positions)

    in_maps = []
    for b in range(B):
        xT_b = _bf16(x[b].T)  # [DM, S]
        for g in range(4):
            rows = slice(g * 256, (g + 1) * 256)
            in_maps.append(
                {
                    "xT": xT_b,
                    "wq_t": _bf16(wq[rows].T),
                    "wk_t": _bf16(wk[rows].T),
                    "wv_t": _bf16(wv[rows].T),
                    "wo_t": _bf16(wo[:, rows].T),
                    "cosT": cosT,
                    "sinT": sinT,
                    "perm": perm,
                    "tri": tri,
                }
            )

    res = run_bass_kernel_spmd(
        nc,
        in_maps,
        core_ids=list(range(8)),
        trace=bool(os.environ.get("BASS_TRACE")),
    )
    _LAST_RESULTS = res
    outs = res.results

    out = np.zeros((B, S, DM), dtype=np.float32)
    for b in range(B):
        for g in range(4):
            out[b] += np.asarray(
                outs[b * 4 + g]["out_partial"], dtype=np.float32
            )
    return out


# revision 12
# speedup vs baseline: 1.0258x; 1.0258x over previous
"""Causal multi-head self-attention with RoPE on 8 Trainium2 NeuronCores.

Problem: x[2, 2048, 1024] fp32, 16 heads, d_head=64, causal, RoPE(theta=1e4).
Sharding: core = b*4 + g  (b in {0,1} batch, g in {0..3} head-group of 4 heads).
Each core computes out_partial[2048, 1024] = attn(heads of g) @ wo[:, cols_g].T
in bf16; host sums the 4 partials per batch in fp32.

v2 layout (vs v1): multi-queue input DMA; ko-outer Q projection pipelined
against the sequential x-chunk arrivals; qc-outer main loop with the output
projection fused per q-chunk; causal narrowing of scores/exp/attn on diagonal
k-tiles; bf16 output; denominator broadcast bounce on the gpsimd DMA queue.
"""

import os
import sys

sys.path.insert(0, "/opt/trn_rl_repo")

import ml_dtypes
import numpy as np

import concourse.bacc as bacc
import concourse.mybir as mybir
from concourse.tile import TileContext

B = 2
S = 2048
DM = 1024
H = 16
DH = 64
HLOC = 4  # heads per core
SC = 512  # q chunk size
NKT = S // 128  # 16 k tiles
NQC = S // SC  # 4 q chunks
P = 128
KO = DM // P  # 8 contraction subtiles for projections
SCALE = 1.0 / 8.0  # 1/sqrt(DH)
THETA = 10000.0

F32 = mybir.dt.float32
BF16 = mybir.dt.bfloat16

_CACHE = {}
DEBUG = False


def _build_nc():
    nc = bacc.Bacc("TRN2", enable_partition_id=False)
    Exp = mybir.ActivationFunctionType.Exp

    xT = nc.dram_tensor("xT", [DM, S], BF16, kind="ExternalInput")
    wq_t = nc.dram_tensor("wq_t", [DM, 256], BF16, kind="ExternalInput")
    wk_t = nc.dram_tensor("wk_t", [DM, 256], BF16, kind="ExternalInput")
    wv_t = nc.dram_tensor("wv_t", [DM, 256], BF16, kind="ExternalInput")
    wo_t = nc.dram_tensor("wo_t", [256, DM], BF16, kind="ExternalInput")
    cosT = nc.dram_tensor("cosT", [P, S], BF16, kind="ExternalInput")
    sinT = nc.dram_tensor("sinT", [P, S], BF16, kind="ExternalInput")
    perm = nc.dram_tensor("perm", [P, P], BF16, kind="ExternalInput")
    tri = nc.dram_tensor("tri", [P, 2, P], BF16, kind="ExternalInput")
    outp = nc.dram_tensor("out_partial", [S, DM], BF16, kind="ExternalOutput")

    with TileContext(nc) as tc:
        with tc.tile_pool(name="persist", bufs=1) as persist, \
             tc.tile_pool(name="mw", bufs=1) as mw, \
             tc.tile_pool(name="mp", bufs=6) as mp, \
             tc.tile_pool(name="mn", bufs=2) as mn, \
             tc.tile_pool(name="mo", bufs=2) as mo, \
             tc.tile_pool(name="mdr", bufs=4, space="DRAM") as mdr:
            # [pair-head-dim (2*64), head-pair, seq]
            q_rot = persist.tile([P, 2, S], BF16, tag="q_rot")
            k_rot = persist.tile([P, 2, S], BF16, tag="k_rot")
            # V in [k partitions, k_tile, head, 72]: cols 0:64 = V, 64 = ones
            v_sb = persist.tile([P, NKT, HLOC, 72], BF16, tag="v_sb")
            # attention output, transposed: [head-dim rows, head-pair, seq]
            attnT = persist.tile([P, 2, S], BF16, tag="attnT")

            wo_sb = mw.tile([P, 2, DM], BF16, tag="wo_sb")
            tri_sb = mw.tile([P, 2, P], BF16, tag="tri_sb")
            # x + projection weights stay resident: projection chains for
            # chunk qc+1 run inside chunk qc's attention window
            xT_sb = mw.tile([P, KO, S], BF16, tag="xT_sb")
            wq_sb = mw.tile([P, KO, 256], BF16, tag="wq_sb")
            wk_sb = mw.tile([P, KO, 256], BF16, tag="wk_sb")
            wv_sb = mw.tile([P, KO, 256], BF16, tag="wv_sb")
            cos_sb = mw.tile([P, S], BF16, tag="cos_sb")
            sin_sb = mw.tile([P, S], BF16, tag="sin_sb")
            perm_sb = mw.tile([P, P], BF16, tag="perm_sb")

            # weights + tables on the scalar engine's DMA queue (per-ko
            # chunks so wq[ko0] lands before the first projection round)
            for t, dt_ in ((wq_sb, wq_t), (wk_sb, wk_t), (wv_sb, wv_t)):
                d_ap = dt_[:].rearrange("(ko p) m -> p ko m", p=P)
                for ko in range(KO):
                    nc.scalar.dma_start(t[:, ko, :], d_ap[:, ko, :])
            nc.scalar.dma_start(cos_sb[:], cosT[:])
            nc.scalar.dma_start(sin_sb[:], sinT[:])
            nc.scalar.dma_start(perm_sb[:], perm[:])
            # x chunks sequentially on the sync queue: chunk ko arrives
            # ~1.6us apart, pacing the ko-outer projection rounds
            xT_ap = xT[:].rearrange("(ko p) s -> p ko s", p=P)
            for ko in range(KO):
                nc.sync.dma_start(xT_sb[:, ko, :], xT_ap[:, ko, :])
            nc.sync.dma_start(
                wo_sb[:], wo_t[:].rearrange("(ko p) m -> p ko m", p=P)
            )
            nc.sync.dma_start(tri_sb[:], tri[:])

            # ones column for the denominator trick
            nc.vector.memset(v_sb[:, :, :, 64:65], 1.0)

            def rope(pool, tag, a_ps, dest, cs_):
                # dest = a*cos + P@(a*sin); a staged to bf16 once on ACT.
                # b_ps reuses the slot a_ps occupied (freed by the copy).
                a_sb = mn.tile([P, SC], BF16, tag="a_sb", bufs=3)
                nc.scalar.copy(out=a_sb[:], in_=a_ps[:])
                t2 = mn.tile([P, SC], BF16, tag="t2", bufs=2)
                nc.vector.tensor_mul(
                    out=t2[:], in0=a_sb[:], in1=sin_sb[:, cs_]
                )
                b_ps = pool.tile([P, SC], F32, tag=tag, name="b_ps")
                nc.tensor.matmul(
                    b_ps[:], lhsT=perm_sb[:], rhs=t2[:],
                    start=True, stop=True,
                )
                nc.vector.tensor_mul(
                    out=dest, in0=a_sb[:], in1=cos_sb[:, cs_]
                )
                nc.vector.tensor_add(out=dest, in0=dest, in1=b_ps[:])

            def v_copy(v_ps, st):
                nc.vector.tensor_copy(
                    out=v_sb[:, st, :, 0:64],
                    in_=v_ps[:].rearrange("p (h d) -> p h d", d=DH),
                )

            # ---------------- Phase B: chunk-0 projections ------------------
            # Only what attention chunk 0 needs: Q/K of qc0 (both hp) and
            # V st0-3, ko-outer across 8 PSUM tiles so the PE follows the
            # x-chunk DMA stream; later chunks project inside the main loop.
            cs0 = slice(0, SC)
            with tc.tile_pool(name="bp", bufs=8, space="PSUM") as bp:
                qa = [bp.tile([P, SC], F32, tag="proj", name=f"qa{hp}")
                      for hp in range(2)]
                ka = [bp.tile([P, SC], F32, tag="proj", name=f"ka{hp}")
                      for hp in range(2)]
                va = [bp.tile([P, 256], F32, tag="proj", name=f"va{st}")
                      for st in range(4)]
                for ko in range(KO):
                    for hp in range(2):
                        nc.tensor.matmul(
                            qa[hp][:],
                            lhsT=wq_sb[:, ko, hp * P:(hp + 1) * P],
                            rhs=xT_sb[:, ko, cs0],
                            start=(ko == 0), stop=(ko == KO - 1),
                        )
                    for hp in range(2):
                        nc.tensor.matmul(
                            ka[hp][:],
                            lhsT=wk_sb[:, ko, hp * P:(hp + 1) * P],
                            rhs=xT_sb[:, ko, cs0],
                            start=(ko == 0), stop=(ko == KO - 1),
                        )
                    for st in range(4):
                        nc.tensor.matmul(
                            va[st][:],
                            lhsT=xT_sb[:, ko, st * P:(st + 1) * P],
                            rhs=wv_sb[:, ko, :],
                            start=(ko == 0), stop=(ko == KO - 1),
                        )
                for hp in range(2):
                    rope(bp, "proj", qa[hp], q_rot[:, hp, cs0], cs0)
                for hp in range(2):
                    rope(bp, "proj", ka[hp], k_rot[:, hp, cs0], cs0)
                for st in range(4):
                    v_copy(va[st], st)

            # ---------------- Main loop: attention + proj + out proj --------
            # hp streams interleave at (kp, j) granularity; next chunk's
            # projection chains slot into the ACT-bound stretches, borrowing
            # the scores ring; phase E o_ps borrows the at rings.
            with tc.tile_pool(name="ds", bufs=2, space="PSUM") as dsp, \
                 tc.tile_pool(name="da", bufs=2, space="PSUM") as dap:
                out_ap = outp[:].rearrange("(st p) m -> p st m", p=P)

                def emit_proj(item, qn):
                    csn = slice(qn * SC, (qn + 1) * SC)
                    kind, i = item
                    if kind == "q" or kind == "k":
                        w_sb = wq_sb if kind == "q" else wk_sb
                        dest = q_rot if kind == "q" else k_rot
                        a_ps = dsp.tile([P, SC], F32, tag="scores",
                                        name="a_ps")
                        for ko in range(KO):
                            nc.tensor.matmul(
                                a_ps[:],
                                lhsT=w_sb[:, ko, i * P:(i + 1) * P],
                                rhs=xT_sb[:, ko, csn],
                                start=(ko == 0), stop=(ko == KO - 1),
                            )
                        rope(dsp, "scores", a_ps, dest[:, i, csn], csn)
                    else:
                        v_ps = dsp.tile([P, 256], F32, tag="scores",
                                        name="v_ps")
                        for ko in range(KO):
                            nc.tensor.matmul(
                                v_ps[:],
                                lhsT=xT_sb[:, ko, i * P:(i + 1) * P],
                                rhs=wv_sb[:, ko, :],
                                start=(ko == 0), stop=(ko == KO - 1),
                            )
                        v_copy(v_ps, i)

                for qc in range(NQC):
                    cs = slice(qc * SC, (qc + 1) * SC)
                    nkt_v = 4 * qc + 4
                    if qc < 3:
                        proj_items = [
                            ("q", 0), ("k", 0), ("q", 1), ("k", 1),
                        ] + [("v", st)
                             for st in range(4 * qc + 4, 4 * qc + 8)]
                    else:
                        proj_items = []
                    n_js = nkt_v * 2
                    stride = max(1, n_js // 8)
                    ji = 0
                    at = {}
                    for hp in range(2):
                        for hh in range(2):
                            at[(hp, hh)] = dap.tile(
                                [65, SC], F32, tag=f"at{hh}",
                                name=f"at{hp}{hh}")
                    def emit_at(hp, kt, pt):
                        for hh in range(2):
                            nc.tensor.matmul(
                                at[(hp, hh)][:],
                                lhsT=v_sb[:, kt, 2 * hp + hh, 0:65],
                                rhs=pt[:, hh, :],
                                start=(kt == 0),
                                stop=(kt == nkt_v - 1),
                            )

                    pending = {0: None, 1: None}
                    for kp in range(nkt_v // 2):
                        for hp in range(2):
                            for j in range(2):
                                if proj_items and ji % stride == 0:
                                    emit_proj(proj_items.pop(0), qc + 1)
                                ji += 1
                                kt = 2 * kp + j
                                r = kt - 4 * qc
                                w0 = 128 * r if r > 0 else 0
                                s2 = dsp.tile([P, 2, SC], F32, tag="scores",
                                              name="s2")
                                for hh in range(2):
                                    hs = slice(hh * 64, (hh + 1) * 64)
                                    nc.tensor.matmul(
                                        s2[:, hh, w0:SC],
                                        lhsT=k_rot[hs, hp,
                                                   kt * P:(kt + 1) * P],
                                        rhs=q_rot[hs, hp,
                                                  qc * SC + w0:(qc + 1) * SC],
                                        start=True,
                                        stop=True,
                                    )
                                pt = mp.tile([P, 2, SC], BF16, tag="pt",
                                             name="pt", bufs=6)
                                if w0 > 0:
                                    nc.gpsimd.memset(pt[:, :, 0:w0], 0.0)
                                nc.scalar.activation(
                                    out=pt[:, :, w0:SC],
                                    in_=s2[:, :, w0:SC],
                                    func=Exp, scale=SCALE,
                                )
                                if r >= 0:
                                    nc.vector.tensor_mul(
                                        out=pt[:, :, w0:w0 + 128],
                                        in0=pt[:, :, w0:w0 + 128],
                                        in1=tri_sb[:],
                                    )
                                # sink this j's attn matmuls one iteration so
                                # the PE never blocks on the exp it just fed
                                if pending[hp] is not None:
                                    emit_at(hp, *pending[hp])
                                pending[hp] = (kt, pt)
                    for hp in range(2):
                        emit_at(hp, *pending[hp])
                    while proj_items:
                        emit_proj(proj_items.pop(0), qc + 1)
                    # normalize: rows 0:64 are attn, row 64 is denom;
                    # partition-broadcast 1/denom via DRAM bounce
                    for hp in range(2):
                        for hh in range(2):
                            rd = mn.tile([P, SC], F32, tag="rd")
                            nc.vector.tensor_copy(
                                out=rd[64:65, :], in_=at[(hp, hh)][64:65, :]
                            )
                            dr = mdr.tile([1, SC], F32, tag="dr")
                            nc.scalar.dma_start(dr[:], rd[64:65, :])
                            den_bc = mn.tile([64, SC], F32, tag="den_bc")
                            nc.scalar.dma_start(
                                den_bc[:], dr[:].partition_broadcast(64)
                            )
                            rbc = mn.tile([64, SC], F32, tag="rbc")
                            nc.vector.reciprocal_approx_fast(
                                out=rbc[:], in_=den_bc[:]
                            )
                            if hh == 0:
                                nc.vector.tensor_mul(
                                    out=attnT[0:64, hp, cs],
                                    in0=at[(hp, hh)][0:64, :],
                                    in1=rbc[:],
                                )
                            else:
                                tmp = mn.tile([64, SC], BF16, tag="tmp")
                                nc.vector.tensor_mul(
                                    out=tmp[:], in0=at[(hp, hh)][0:64, :],
                                    in1=rbc[:],
                                )
                                nc.scalar.dma_start(
                                    attnT[64:128, hp, cs], tmp[:]
                                )
                    # output projection for this q chunk; o_ps borrows the
                    # at-tag rings so next chunk's scores never wait on it
                    for st in range(4 * qc, 4 * qc + 4):
                        ob = mo.tile([P, DM], BF16, tag="ob")
                        for no in range(2):
                            o_ps = dap.tile([P, SC], F32, tag=f"at{no}",
                                            name="o_ps")
                            for ko in range(2):
                                nc.tensor.matmul(
                                    o_ps[:],
                                    lhsT=attnT[:, ko, st * P:(st + 1) * P],
                                    rhs=wo_sb[:, ko, no * SC:(no + 1) * SC],
                                    start=(ko == 0),
                                    stop=(ko == 1),
                                )
                            osl = ob[:, no * SC:(no + 1) * SC]
                            nc.vector.tensor_copy(out=osl, in_=o_ps[:])
                        nc.sync.dma_start(out_ap[:, st, :], ob[:])
            if DEBUG:
                dq = nc.dram_tensor("dbg_qrot", [P, 2, S], BF16,
                                    kind="ExternalOutput")
                dk = nc.dram_tensor("dbg_krot", [P, 2, S], BF16,
                                    kind="ExternalOutput")
                dv = nc.dram_tensor("dbg_vsb", [P, NKT, HLOC, 72], BF16,
                                    kind="ExternalOutput")
                da_ = nc.dram_tensor("dbg_attnT", [P, 2, S], BF16,
                                     kind="ExternalOutput")
                nc.sync.dma_start(dq[:], q_rot[:])
                nc.sync.dma_start(dk[:], k_rot[:])
                nc.sync.dma_start(dv[:], v_sb[:])
                nc.sync.dma_start(da_[:], attnT[:])
    nc.compile()
    return nc


def _host_tables(token_positions):
    pos = np.asarray(token_positions).astype(np.float64)
    freq = 1.0 / (THETA ** (2.0 * np.arange(DH // 2, dtype=np.float64) / DH))
    ang = pos[:, None] * freq[None, :]  # [S, 32]
    cos_f = np.repeat(np.cos(ang), 2, axis=1)  # [S, 64]
    sin_f = np.repeat(np.sin(ang), 2, axis=1)
    cosT = np.ascontiguousarray(
        np.concatenate([cos_f.T, cos_f.T], axis=0)
    ).astype(ml_dtypes.bfloat16)  # [128, S]
    sinT = np.ascontiguousarray(
        np.concatenate([sin_f.T, sin_f.T], axis=0)
    ).astype(ml_dtypes.bfloat16)

    perm = np.zeros((P, P), dtype=ml_dtypes.bfloat16)
    for i in range(P // 2):
        perm[2 * i + 1, 2 * i] = -1.0
        perm[2 * i, 2 * i + 1] = 1.0

    p_idx = np.arange(P)[:, None]
    f_idx = np.arange(P)[None, :]
    tri1 = (f_idx >= p_idx).astype(ml_dtypes.bfloat16)  # [128, 128]
    tri = np.ascontiguousarray(
        np.broadcast_to(tri1[:, None, :], (P, 2, P))
    )  # [128, 2, 128]
    return cosT, sinT, perm, tri


_LAST_RESULTS = None


def _bf16(a):
    return np.ascontiguousarray(a).astype(ml_dtypes.bfloat16)


def kernel(x, wq, wk, wv, wo, token_positions):
    global _LAST_RESULTS
    from concourse.bass_utils import run_bass_kernel_spmd

    if "nc" not in _CACHE:
        _CACHE["nc"] = _build_nc()
    nc = _CACHE["nc"]

    x = np.asarray(x, dtype=np.float32)
    wq = np.asarray(wq, dtype=np.float32)
    wk = np.asarray(wk, dtype=np.float32)
    wv = np.asarray(wv, dtype=np.float32)
    wo = np.asarray(wo, dtype=np.float32)
    cosT, sinT, perm, tri = _host_tables(token_positions)

    in_maps = []
    for b in range(B):
        xT_b = _bf16(x[b].T)  # [DM, S]
        for g in range(4):
            rows = slice(g * 256, (g + 1) * 256)
            in_maps.append(
                {
                    "xT": xT_b,
                    "wq_t": _bf16(wq[rows].T),
                    "wk_t": _bf16(wk[rows].T),
                    "wv_t": _bf16(wv[rows].T),
                    "wo_t": _bf16(wo[:, rows].T),
                    "cosT": cosT,
                    "sinT": sinT,
                    "perm": perm,
                    "tri": tri,
                }
            )

    res = run_bass_kernel_spmd(
        nc,
        in_maps,
        core_ids=list(range(8)),
        trace=bool(os.environ.get("BASS_TRACE")),
    )
    _LAST_RESULTS = res
    outs = res.results

    out = np.zeros((B, S, DM), dtype=np.float32)
    for b in range(B):
        for g in range(4):
            out[b] += np.asarray(
                outs[b * 4 + g]["out_partial"], dtype=np.float32
            )
    return out
